# revision 1
# baseline (speedup 1.0000x reference)
"""Cross-attention Trainium2 kernel (B=8, T=1024, S=1500, D=1024, H=16, Dh=64).

Sharding: pure data-parallel on batch — core b computes batch b end to end
(no collectives). Per-core pipeline, all operands SBUF-resident in bf16.

The kernel is ACT-bound: softmax needs exp of H*T*S scores and ScalarE is the
only exp engine at 1 elem/lane/cycle, ~197us/core at FD=1536 per ACTIVATE.
Everything else is scheduled to hide under that stream:

  - each head pair is processed in two T-half phases; per phase the 24
    score quarters [128s x 512t] pack into eight [128,1536] PSUM tiles
    (3 banks x 2 slots) so each exp amortizes the ~290ns ACT setup over
    1536 columns
  - PV accumulates per phase into [*,512] PSUM tiles (1 bank x 2 slots,
    ones-column trick carries the softmax denominator in row 64)
  - q_proj for the next pair's e-tile runs as two 8-matmul bursts in the
    PV slots at phase boundaries, so the score slots feeding ACT are
    never stolen
  - out_proj drains per f-tile with the store DMA overlapped

Host side pre-transposes/casts inputs, pre-blocks wq/wo per out-tile, and
transposes the [f,t] output back.
"""

import sys

for _p in ("/opt/trn_rl_repo", "/root/.axon_site/_ro/trn_rl_repo"):
    if _p not in sys.path:
        sys.path.insert(0, _p)

import numpy as np
import ml_dtypes

import concourse.bass as bass
import concourse.mybir as mybir
import concourse.tile as tile
from concourse import bacc
from concourse import bass_utils

BF16 = ml_dtypes.bfloat16

P = 128
B = 8
T = 1024
S0 = 1500          # real source length
S = 1536           # padded to 12*128
D = 1024
H = 16
Dh = 64
DT = D // P        # 8 d/e/f tiles
ST = S // P        # 12 s chunks
NPAIR = H // 2     # 8 head pairs
HW = Dh + 1        # 65: per-head v width incl. ones column
SCALE = Dh ** -0.5
TH = 512           # T-half width (phase granularity)
FQ = 24            # score quarters per phase (ST * 2 heads)
NT = FQ // 3       # 8 [128,1536] exp tiles per phase

f32 = mybir.dt.float32
bf16 = mybir.dt.bfloat16


def build_bass():
    nc = bacc.Bacc("TRN2", target_bir_lowering=False, debug=False,
                   enable_asserts=False, num_devices=B)

    xT_d = nc.dram_tensor("xT", [D, T], bf16, kind="ExternalInput")
    kT_d = nc.dram_tensor("kT", [D, S], bf16, kind="ExternalInput")
    va_d = nc.dram_tensor("vaug", [S, H * HW], bf16, kind="ExternalInput")
    # wqb/wob pre-blocked: rows j*128.. hold the eight [128,128] lhsT slabs
    # of out-tile j, so one [128,1024] DMA covers e/f-tile j.
    wqb_d = nc.dram_tensor("wqb", [D, D], bf16, kind="ExternalInput")
    bq_d = nc.dram_tensor("bqr", [P, DT], f32, kind="ExternalInput")
    wob_d = nc.dram_tensor("wob", [D, D], bf16, kind="ExternalInput")
    bo_d = nc.dram_tensor("bor", [P, DT], f32, kind="ExternalInput")
    outT_d = nc.dram_tensor("outT", [D, T], f32, kind="ExternalOutput")

    EXP = mybir.ActivationFunctionType.Exp

    with tile.TileContext(nc) as tc:
        with (
            tc.tile_pool(name="const", bufs=1) as cp,
            tc.tile_pool(name="work", bufs=2) as wp,
            tc.tile_pool(name="psum_mm", bufs=2, space="PSUM") as mmp,
            tc.tile_pool(name="psum_pv", bufs=2, space="PSUM") as pvp,
        ):
            def load1(dram, cols, j, tagbase, dt=bf16):
                t = cp.tile([P, cols], dt, name=f"{tagbase}{j}",
                            tag=f"{tagbase}{j}")
                nc.sync.dma_start(t[:], dram[j * P:(j + 1) * P, :])
                return t

            # warm the ACT exp table while DMAs stream
            dummy = cp.tile([1, 8], f32, name="dummy", tag="dummy")
            nc.vector.memset(dummy[:], 0.0)
            nc.scalar.activation(dummy[:], dummy[:], EXP)

            # xT as per-half tiles: q_proj phase ph reads only half ph, and
            # separate tiles avoid coarse per-tile false deps on the DMAs.
            def loadx(dt_i, ph):
                t = cp.tile([P, TH], bf16, name=f"xTs{dt_i}_{ph}",
                            tag=f"xTs{dt_i}_{ph}")
                nc.sync.dma_start(
                    t[:], xT_d[dt_i * P:(dt_i + 1) * P,
                               ph * TH:(ph + 1) * TH])
                return t

            # DMA priority order: q_proj(0) half-0 inputs first, then pair-0.
            xTh_sb = {(dt_i, 0): loadx(dt_i, 0) for dt_i in range(DT)}
            wqb_sb = {0: load1(wqb_d, D, 0, "wqbs")}
            kT_sb = {0: load1(kT_d, S, 0, "kTs")}
            xTh_sb.update({(dt_i, 1): loadx(dt_i, 1) for dt_i in range(DT)})
            wqb_sb[1] = load1(wqb_d, D, 1, "wqbs")
            bq_sb = cp.tile([P, DT], f32, name="bq_sb", tag="bq_sb")
            nc.sync.dma_start(bq_sb[:], bq_d[:, :])
            va_sb = {c: load1(va_d, H * HW, c, "vas") for c in range(4)}
            kT_sb[1] = load1(kT_d, S, 1, "kTs")
            va_sb.update({c: load1(va_d, H * HW, c, "vas")
                          for c in range(4, ST)})
            wqb_sb.update({j: load1(wqb_d, D, j, "wqbs")
                           for j in range(2, DT)})
            kT_sb.update({j: load1(kT_d, S, j, "kTs") for j in range(2, DT)})
            wob_sb = [load1(wob_d, D, j, "wobs") for j in range(DT)]
            bo_sb = cp.tile([P, DT], f32, name="bo_sb", tag="bo_sb")
            nc.sync.dma_start(bo_sb[:], bo_d[:, :])

            # qT/aT as per-half tiles for the same reason: the deferred
            # q_proj evict writing half 1 must not serialize score matmuls
            # reading half 0.
            qTh_sb = {(j, ph): cp.tile([P, TH], bf16, name=f"qTs{j}_{ph}",
                                       tag=f"qTs{j}_{ph}")
                      for j in range(DT) for ph in range(2)}
            aTh_sb = {(j, ph): cp.tile([P, TH], bf16, name=f"aTs{j}_{ph}",
                                       tag=f"aTs{j}_{ph}")
                      for j in range(DT) for ph in range(2)}

            # ---- q projection for e-tile j, one T-half (8 MMs + evict) ----
            def qproj_mms(qp, j, ph, dts, evict):
                for dt_i in dts:
                    nc.tensor.matmul(
                        qp[:, :],
                        lhsT=wqb_sb[j][:, dt_i * P:(dt_i + 1) * P],
                        rhs=xTh_sb[(dt_i, ph)][:, :],
                        start=(dt_i == 0), stop=(dt_i == DT - 1),
                    )
                if evict:
                    nc.vector.tensor_scalar_add(qTh_sb[(j, ph)][:, :],
                                                qp[:, :], bq_sb[:, j:j + 1])

            def qproj_half(j, ph):
                qp = pvp.tile([P, TH], f32, name=f"qp{j}_{ph}", tag="pv")
                qproj_mms(qp, j, ph, range(DT), True)

            # prologue: qT[0]
            qproj_half(0, 0)
            qproj_half(0, 1)

            # ---- attention: pair j, phase ph covers t-half ph --------------
            # The next pair's q_proj burst gets its PSUM slot at the phase
            # boundary (allocation order) but its matmuls are emitted two
            # exp-tiles into the following phase so the PE never delays the
            # score tiles that feed the (bottleneck) ACT stream.
            def emit_sc_exp(j, ph, k):
                sct = mmp.tile([P, 3 * TH], f32, name=f"sc{j}{ph}{k}",
                               tag="mm")
                for q in range(3):
                    qi = 3 * k + q
                    c, a = qi // 2, qi % 2
                    rows = slice(a * Dh, (a + 1) * Dh)
                    nc.tensor.matmul(
                        sct[:, q * TH:(q + 1) * TH],
                        lhsT=kT_sb[j][rows, c * P:(c + 1) * P],
                        rhs=qTh_sb[(j, ph)][rows, :],
                        start=True, stop=True,
                    )
                pt = wp.tile([P, 3 * TH], bf16, name=f"pt{j}{ph}{k}",
                             tag="pt", bufs=8)
                nc.scalar.activation(pt[:, :], sct[:, :], EXP)
                return pt

            def emit_pv(j, ph, k, pt, pv):
                for q in range(3):
                    qi = 3 * k + q
                    c, a = qi // 2, qi % 2
                    h = 2 * j + a
                    nc.tensor.matmul(
                        pv[a][0:HW, :],
                        lhsT=va_sb[c][:, h * HW:(h + 1) * HW],
                        rhs=pt[:, q * TH:(q + 1) * TH],
                        start=(c == 0), stop=(c == ST - 1),
                    )

            # the next phase's first score tile + exp are emitted BEFORE the
            # boundary work (pv evicts / q_proj burst / normalize) so the ACT
            # stream never drains across a phase boundary; only its PV
            # matmuls are deferred to after the new accumulators exist.
            head_pt = None
            for j in range(NPAIR):
                for ph in range(2):
                    pv = [pvp.tile([P, TH], f32, name=f"pv{j}_{ph}_{a}",
                                   tag="pv") for a in range(2)]
                    if head_pt is None:
                        head_pt = emit_sc_exp(j, ph, 0)
                    emit_pv(j, ph, 0, head_pt, pv)
                    for k in range(1, NT):
                        pt = emit_sc_exp(j, ph, k)
                        emit_pv(j, ph, k, pt, pv)
                    head_pt = None
                    if (j, ph) != (NPAIR - 1, 1):
                        nj, nph = (j, 1) if ph == 0 else (j + 1, 0)
                        head_pt = emit_sc_exp(nj, nph, 0)
                    # evict both heads' pv, freeing the slots for the next
                    # phase's q_proj burst / accumulators; the last pair
                    # evicts on ACT (idle once the exps are done)
                    pvsb = []
                    for a in range(2):
                        sb = wp.tile([HW, TH], f32, name=f"pvsb{j}{ph}{a}",
                                     tag="pvsb", bufs=4)
                        if (j, ph) == (NPAIR - 1, 1):
                            nc.scalar.copy(sb[:, :], pv[a][0:HW, :])
                        else:
                            nc.vector.tensor_copy(sb[:, :], pv[a][0:HW, :])
                        pvsb.append(sb)
                    if j + 1 < NPAIR:
                        qproj_half(j + 1, ph)
                    # normalize: attnT = pv[0:64] / pv[64] (DVE + gpsimd)
                    for a in range(2):
                        sb = pvsb[a]
                        dsm = wp.tile([Dh, TH // Dh], f32,
                                      name=f"ds{j}{ph}{a}", tag="dsm", bufs=4)
                        nc.sync.dma_start(dsm[:, :], sb[Dh:Dh + 1, :])
                        nc.vector.reciprocal(dsm[:, :], dsm[:, :])
                        rrow = wp.tile([1, TH], f32, name=f"rr{j}{ph}{a}",
                                       tag="rrow", bufs=4)
                        nc.sync.dma_start(rrow[:, :], dsm[:, :])
                        nrm = wp.tile([Dh, TH], f32, name=f"nr{j}{ph}{a}",
                                      tag="nrm", bufs=4)
                        nc.gpsimd.partition_broadcast(nrm[:, :], rrow[0:1, :])
                        nc.vector.tensor_mul(
                            aTh_sb[(j, ph)][a * Dh:(a + 1) * Dh, :],
                            sb[0:Dh, :], nrm[:, :])

            # ---- out projection  outT[f,t] ---------------------------------
            for fj in range(DT):
                ps = mmp.tile([P, T], f32, name=f"op{fj}", tag="mm")
                for tch in range(2):
                    tsl = slice(tch * TH, (tch + 1) * TH)
                    for et in range(DT):
                        nc.tensor.matmul(
                            ps[:, tsl],
                            lhsT=wob_sb[fj][:, et * P:(et + 1) * P],
                            rhs=aTh_sb[(et, tch)][:, :],
                            start=(et == 0), stop=(et == DT - 1),
                        )
                for tch in range(2):
                    tsl = slice(tch * TH, (tch + 1) * TH)
                    ost = wp.tile([P, TH], f32, name=f"ost{fj}_{tch}",
                                  tag="ost", bufs=3)
                    nc.vector.tensor_scalar_add(ost[:, :], ps[:, tsl],
                                                bo_sb[:, fj:fj + 1])
                    nc.sync.dma_start(outT_d[fj * P:(fj + 1) * P, tsl],
                                      ost[:, :])

    nc.compile()
    return nc


def prep_inputs(x, k, v, wq, bq, wo, bo):
    """Host-side shard + layout prep. Returns per-core in_maps."""
    x = np.asarray(x, np.float32)
    k = np.asarray(k, np.float32)
    v = np.asarray(v, np.float32)
    wq = np.asarray(wq, np.float32)
    bq = np.asarray(bq, np.float32)
    wo = np.asarray(wo, np.float32)
    bo = np.asarray(bo, np.float32)

    wqT = np.ascontiguousarray((wq * SCALE).T).astype(BF16)       # [d, e]
    woT = np.ascontiguousarray(wo.T).astype(BF16)                 # [e, f]
    wqb = np.zeros((D, D), BF16)
    wob = np.zeros((D, D), BF16)
    for j in range(DT):
        for dt_i in range(DT):
            wqb[j * P:(j + 1) * P, dt_i * P:(dt_i + 1) * P] = \
                wqT[dt_i * P:(dt_i + 1) * P, j * P:(j + 1) * P]
            wob[j * P:(j + 1) * P, dt_i * P:(dt_i + 1) * P] = \
                woT[dt_i * P:(dt_i + 1) * P, j * P:(j + 1) * P]
    bqr = np.ascontiguousarray((bq * SCALE).reshape(DT, P).T)     # [P, DT]
    bor = np.ascontiguousarray(bo.reshape(DT, P).T)               # [P, DT]

    in_maps = []
    for b in range(x.shape[0]):
        xT = np.ascontiguousarray(x[b].T).astype(BF16)            # [D, T]
        kT = np.zeros((D, S), BF16)
        kT[:, :S0] = k[b].T.astype(BF16)
        vaug = np.zeros((S, H * HW), BF16)
        vb = v[b].astype(BF16)
        for h in range(H):
            vaug[:S0, h * HW:h * HW + Dh] = vb[:, h * Dh:(h + 1) * Dh]
            vaug[:S0, h * HW + Dh] = BF16(1.0)
        in_maps.append({
            "xT": xT, "kT": kT, "vaug": np.ascontiguousarray(vaug),
            "wqb": wqb, "bqr": bqr, "wob": wob, "bor": bor,
        })
    return in_maps


_NC_CACHE = {}


def kernel(x, k, v, wq, bq, wo, bo, _trace=False):
    if "nc" not in _NC_CACHE:
        _NC_CACHE["nc"] = build_bass()
    nc = _NC_CACHE["nc"]
    in_maps = prep_inputs(x, k, v, wq, bq, wo, bo)
    res = bass_utils.run_bass_kernel_spmd(
        nc, in_maps, core_ids=list(range(B)), trace=_trace)
    _NC_CACHE["last_result"] = res
    out = np.stack([np.ascontiguousarray(r["outT"].T) for r in res.results])
    return out

